# revision 6
# baseline (speedup 1.0000x reference)
"""Trainium2 Bass kernel for nn_CumsumEmbedding.

Computes: LayerNorm(cumsum(embedding_matrix, axis=0))[index_tensor]

Strategy (pure data parallel over 8 NeuronCores):
  - Each core receives the full [8192,128] embedding table (replicated), its
    own 1/8 shard of the indices ([8,8192] of the [64,8192] batch), plus small
    constants.
  - On-device table prep: within-tile (128-row) prefix sums via a triangular
    matmul on the PE, tile carries via a second small triangular matmul,
    LayerNorm fused out of PSUM, normed table written to a DRAM scratch.
  - Gather: SWDGE dma_gather pulls 512B rows from the DRAM normed table into
    SBUF in 8192-index chunks; indices are pre-permuted on the host so the
    SBUF->DRAM writeback is one fully contiguous 4MB DMA per chunk.
"""

import numpy as np

import concourse.bacc as bacc
import concourse.bass as bass
import concourse.tile as tile
from concourse import mybir
from concourse.bass_utils import run_bass_kernel_spmd
from concourse.library_config import mlp

F32 = mybir.dt.float32
I16 = mybir.dt.int16

N = 8192      # table rows
D = 128       # embed dim
B, S = 64, 8192
NCORES = 8
ROWS_PER_CORE = B * S // NCORES          # 65536
CHUNK = 8192                             # indices per dma_gather
NCHUNK = ROWS_PER_CORE // CHUNK          # 8
NT = N // 128                            # 64 tiles of 128 rows
NJ = NT // 4                             # 16 psum chunks of 4 tiles (N=512)
LN_EPS = 1e-5


def _build_program():
    nc = bacc.Bacc(
        "TRN2",
        target_bir_lowering=False,
        debug=False,
        enable_asserts=False,
        num_devices=NCORES,
    )

    emb = nc.dram_tensor("emb", [N, D], F32, kind="ExternalInput").ap()
    gamma = nc.dram_tensor("gamma", [1, D], F32, kind="ExternalInput").ap()
    beta = nc.dram_tensor("beta", [1, D], F32, kind="ExternalInput").ap()
    idx = nc.dram_tensor("idx", [128, NCHUNK * CHUNK // 16], I16,
                         kind="ExternalInput").ap()
    triu = nc.dram_tensor("triu", [128, 128], F32, kind="ExternalInput").ap()
    su64 = nc.dram_tensor("su64", [64, 64], F32, kind="ExternalInput").ap()
    out = nc.dram_tensor("out", [ROWS_PER_CORE, D], F32,
                         kind="ExternalOutput").ap()
    normed = nc.dram_tensor("normed", [N, D], F32).ap()

    with tile.TileContext(nc) as tc:
        with tc.tile_pool(name="consts", bufs=1) as consts, \
             tc.tile_pool(name="sb", bufs=1) as sb, \
             tc.tile_pool(name="stats", bufs=4) as stats, \
             tc.tile_pool(name="ysb", bufs=4) as ysb, \
             tc.tile_pool(name="ps_s", bufs=1, space="PSUM") as ps_s, \
             tc.tile_pool(name="ps_c", bufs=1, space="PSUM") as ps_c, \
             tc.tile_pool(name="ps_p", bufs=6, space="PSUM") as ps_p:

            # gpsimd ucode library for dma_gather; issued first.
            nc.gpsimd.load_library(mlp)

            # ---- constants / inputs to SBUF ----
            triu_sb = consts.tile([128, 128], F32)
            nc.sync.dma_start(out=triu_sb, in_=triu)
            su64_sb = consts.tile([64, 64], F32)
            nc.sync.dma_start(out=su64_sb, in_=su64)
            ones1_sb = consts.tile([1, 128], F32)
            nc.vector.memset(ones1_sb, 1.0)
            eps_sb = consts.tile([128, 1], F32)
            nc.vector.memset(eps_sb, LN_EPS)
            gamma_rep = consts.tile([128, D], F32)
            nc.sync.dma_start(
                out=gamma_rep,
                in_=bass.AP(tensor=gamma.tensor, offset=gamma.offset,
                            ap=[[0, 128], gamma.ap[1]]))
            beta_rep = consts.tile([128, D], F32)
            nc.sync.dma_start(
                out=beta_rep,
                in_=bass.AP(tensor=beta.tensor, offset=beta.offset,
                            ap=[[0, 128], beta.ap[1]]))

            idx_sb = consts.tile([128, NCHUNK, CHUNK // 16], I16)
            nc.sync.dma_start(out=idx_sb, in_=idx)

            # E_sb[p, t, d] = emb[t*128 + p, d]
            E_sb = sb.tile([128, NT, 128], F32)
            nc.sync.dma_start(out=E_sb,
                              in_=emb.rearrange("(t p) d -> p t d", p=128))

            # ---- per-128-row-tile column sums S[t, d] ----
            # S_flat[0, t*128 + d] = sum_p E_sb[p, t, d]
            onescol_sb = consts.tile([128, 1], F32)
            nc.vector.memset(onescol_sb, 1.0)
            E_flat = E_sb.rearrange("p t d -> p (t d)")
            S_flat = sb.tile([1, NT * 128], F32)
            for j in range(NJ):
                S_ps = ps_s.tile([1, 512], F32)
                nc.tensor.matmul(
                    S_ps, onescol_sb,
                    E_flat[:, j * 512:(j + 1) * 512],
                    start=True, stop=True)
                nc.scalar.copy(out=S_flat[:, j * 512:(j + 1) * 512], in_=S_ps)
            # redistribute to S_sb[t, d] = S of tile t
            S_sb = sb.tile([64, 128], F32)
            nc.sync.dma_start(
                out=S_sb.rearrange("t d -> t () d"),
                in_=S_flat.rearrange("p (t d) -> p t d", d=128))

            # ---- carries: carry[t, d] = sum_{s < t} S[s, d] ----
            carry_ps = ps_c.tile([64, 128], F32)
            nc.tensor.matmul(carry_ps, su64_sb, S_sb, start=True, stop=True)
            carry_sb = sb.tile([64, 128], F32)
            nc.scalar.copy(out=carry_sb, in_=carry_ps)
            # flatten to one partition: carry_flat[0, t*128+d] = carry[t, d]
            carry_flat = sb.tile([1, NT * 128], F32)
            nc.sync.dma_start(
                out=carry_flat.rearrange("p (t d) -> p t d", d=128),
                in_=carry_sb.rearrange("t d -> t () d"))

            # ---- main pass: cumsum chunks + LayerNorm -> normed table ----
            normed_t = normed.rearrange("(t p) d -> p t d", p=128)
            for j in range(NJ):
                P = ps_p.tile([128, 4, 128], F32)
                P_flat = P.rearrange("p u d -> p (u d)")
                nc.tensor.matmul(
                    P_flat, triu_sb, E_flat[:, j * 512:(j + 1) * 512],
                    start=True, stop=False)
                nc.tensor.matmul(
                    P_flat, ones1_sb,
                    carry_flat[:, j * 512:(j + 1) * 512],
                    start=False, stop=True)

                st = stats.tile([128, 4, 6], F32)
                mv = stats.tile([128, 4, 2], F32)
                for u in range(4):
                    nc.vector.bn_stats(out=st[:, u, :], in_=P[:, u, :])
                    nc.vector.bn_aggr(out=mv[:, u, :], in_=st[:, u, :])
                # rstd = 1/sqrt(var + eps)
                rstd = stats.tile([128, 4], F32)
                nc.scalar.activation(
                    out=rstd, in_=mv[:, :, 1],
                    func=mybir.ActivationFunctionType.Sqrt,
                    bias=eps_sb, scale=1.0)
                nc.vector.reciprocal(out=rstd, in_=rstd)
                # negmurstd = -mu * rstd
                nmr = stats.tile([128, 4], F32)
                nc.vector.scalar_tensor_tensor(
                    out=nmr, in0=mv[:, :, 0], scalar=-1.0, in1=rstd,
                    op0=mybir.AluOpType.mult, op1=mybir.AluOpType.mult)

                Y = ysb.tile([128, 4, 128], F32)
                for u in range(4):
                    nc.scalar.activation(
                        out=Y[:, u, :], in_=P[:, u, :],
                        func=mybir.ActivationFunctionType.Identity,
                        bias=nmr[:, u:u + 1], scale=rstd[:, u:u + 1])
                # gamma / beta (broadcast across the 4 tiles of the chunk)
                g_b = bass.AP(tensor=gamma_rep.tensor, offset=gamma_rep.offset,
                              ap=[gamma_rep.ap[0], [0, 4], gamma_rep.ap[1]])
                b_b = bass.AP(tensor=beta_rep.tensor, offset=beta_rep.offset,
                              ap=[beta_rep.ap[0], [0, 4], beta_rep.ap[1]])
                nc.vector.tensor_mul(Y, Y, g_b)
                nc.vector.tensor_add(Y, Y, b_b)

                nc.sync.dma_start(out=normed_t[:, 4 * j:4 * j + 4, :], in_=Y)

            # all table writes must land before gathers read the table
            tc.strict_bb_all_engine_barrier()

            # ---- gather phase ----
            out_t = out.rearrange("(g p c) d -> g p c d", g=NCHUNK, p=128)
            with tc.tile_pool(name="gat", bufs=2) as gat:
                for g in range(NCHUNK):
                    G = gat.tile([128, CHUNK // 128, D], F32)
                    nc.gpsimd.dma_gather(
                        G, normed, idx_sb[:, g, :], CHUNK, CHUNK, D,
                        single_packet=False)
                    nc.sync.dma_start(out=out_t[g], in_=G)

    nc.compile()
    return nc


_CACHE = {}


def _get_program():
    if "nc" not in _CACHE:
        _CACHE["nc"] = _build_program()
    return _CACHE["nc"]


def _format_indices(flat_idx: np.ndarray) -> np.ndarray:
    """Per-core index formatting for dma_gather.

    flat_idx: [ROWS_PER_CORE] int array (this core's output rows, in order).
    Returns [128, NCHUNK * CHUNK//16] int16 SBUF image.

    For chunk g (output rows base..base+CHUNK-1):
      - gather list position i = c*128 + p must hold the index for output
        row base + p*(CHUNK//128) + c, so that gathered SBUF[p, c] rows write
        back to DRAM with one fully-contiguous DMA.
      - the SWDGE consumes the list wrapped over 16 partitions
        (idxs[q, s] = list[s*16 + q%16], replicated across the 8 Q7 cores).
    """
    C = CHUNK // 128  # 64
    cols = CHUNK // 16  # 512
    outimg = np.empty((128, NCHUNK * cols), dtype=np.int16)
    for g in range(NCHUNK):
        chunk = flat_idx[g * CHUNK:(g + 1) * CHUNK]
        lst = chunk.reshape(128, C).T.reshape(-1)  # lst[c*128+p]
        w16 = lst.reshape(cols, 16).T.astype(np.int16)  # [16, cols]
        outimg[:, g * cols:(g + 1) * cols] = np.tile(w16, (8, 1))
    return outimg


def kernel(index_tensor, embedding_matrix, ln_gamma, ln_beta):
    index_tensor = np.asarray(index_tensor)
    embedding_matrix = np.asarray(embedding_matrix, dtype=np.float32)
    ln_gamma = np.asarray(ln_gamma, dtype=np.float32).reshape(1, D)
    ln_beta = np.asarray(ln_beta, dtype=np.float32).reshape(1, D)
    idx_dtype = index_tensor.dtype

    triu = np.triu(np.ones((128, 128), dtype=np.float32))
    su64 = np.triu(np.ones((64, 64), dtype=np.float32), k=1)

    nc = _get_program()

    batches_per_core = B // NCORES
    in_maps = []
    for c in range(NCORES):
        shard = index_tensor[c * batches_per_core:(c + 1) * batches_per_core]
        flat = np.ascontiguousarray(shard).reshape(-1).astype(np.int64)
        in_maps.append({
            "emb": embedding_matrix,
            "gamma": ln_gamma,
            "beta": ln_beta,
            "idx": _format_indices(flat),
            "triu": triu,
            "su64": su64,
        })

    res = run_bass_kernel_spmd(nc, in_maps, core_ids=list(range(NCORES)))
    outs = [r["out"].reshape(batches_per_core, S, D) for r in res.results]
    full = np.concatenate(outs, axis=0)
    # keep index dtype untouched on the way out (ints in, floats out)
    del idx_dtype
    return full


# revision 7
# speedup vs baseline: 2.2931x; 2.2931x over previous
"""Trainium2 Bass kernel for nn_CumsumEmbedding.

Computes: LayerNorm(cumsum(embedding_matrix, axis=0))[index_tensor]

Strategy (pure data parallel over 8 NeuronCores):
  - Each core receives the full [8192,128] embedding table (replicated), its
    own 1/8 shard of the indices ([8,8192] of the [64,8192] batch), plus small
    constants.
  - On-device table prep: within-tile (128-row) prefix sums via a triangular
    matmul on the PE, tile carries via a second small triangular matmul,
    LayerNorm fused out of PSUM, normed table written to a DRAM scratch.
  - Gather: SWDGE dma_gather pulls 512B rows from the DRAM normed table into
    SBUF in 8192-index chunks; indices are pre-permuted on the host so the
    SBUF->DRAM writeback is one fully contiguous 4MB DMA per chunk.
"""

import numpy as np

import concourse.bacc as bacc
import concourse.bass as bass
import concourse.tile as tile
from concourse import mybir
from concourse.bass_utils import run_bass_kernel_spmd
from concourse.library_config import mlp

F32 = mybir.dt.float32
I16 = mybir.dt.int16

N = 8192      # table rows
D = 128       # embed dim
B, S = 64, 8192
NCORES = 8
ROWS_PER_CORE = B * S // NCORES          # 65536
CHUNK = 2048                             # indices per dma_gather
NCHUNK = ROWS_PER_CORE // CHUNK          # 32
NT = N // 128                            # 64 tiles of 128 rows
NJ = NT // 4                             # 16 psum chunks of 4 tiles (N=512)
LN_EPS = 1e-5


def _build_program():
    nc = bacc.Bacc(
        "TRN2",
        target_bir_lowering=False,
        debug=False,
        enable_asserts=False,
        num_devices=NCORES,
        num_swdge_queues=4,
    )

    emb = nc.dram_tensor("emb", [N, D], F32, kind="ExternalInput").ap()
    gamma = nc.dram_tensor("gamma", [1, D], F32, kind="ExternalInput").ap()
    beta = nc.dram_tensor("beta", [1, D], F32, kind="ExternalInput").ap()
    idx = nc.dram_tensor("idx", [128, NCHUNK * CHUNK // 16], I16,
                         kind="ExternalInput").ap()
    triu = nc.dram_tensor("triu", [128, 128], F32, kind="ExternalInput").ap()
    su64 = nc.dram_tensor("su64", [64, 64], F32, kind="ExternalInput").ap()
    out = nc.dram_tensor("out", [ROWS_PER_CORE, D], F32,
                         kind="ExternalOutput").ap()
    normed = nc.dram_tensor("normed", [N, D], F32).ap()

    with tile.TileContext(nc) as tc:
        with tc.tile_pool(name="consts", bufs=1) as consts, \
             tc.tile_pool(name="sb", bufs=1) as sb, \
             tc.tile_pool(name="stats", bufs=4) as stats, \
             tc.tile_pool(name="ysb", bufs=4) as ysb, \
             tc.tile_pool(name="ps_s", bufs=1, space="PSUM") as ps_s, \
             tc.tile_pool(name="ps_c", bufs=1, space="PSUM") as ps_c, \
             tc.tile_pool(name="ps_p", bufs=6, space="PSUM") as ps_p:

            # gpsimd ucode library for dma_gather; issued first.
            nc.gpsimd.load_library(mlp)

            # ---- constants / inputs to SBUF ----
            triu_sb = consts.tile([128, 128], F32)
            nc.sync.dma_start(out=triu_sb, in_=triu)
            su64_sb = consts.tile([64, 64], F32)
            nc.sync.dma_start(out=su64_sb, in_=su64)
            ones1_sb = consts.tile([1, 128], F32)
            nc.vector.memset(ones1_sb, 1.0)
            eps_sb = consts.tile([128, 1], F32)
            nc.vector.memset(eps_sb, LN_EPS)
            gamma_rep = consts.tile([128, D], F32)
            nc.sync.dma_start(
                out=gamma_rep,
                in_=bass.AP(tensor=gamma.tensor, offset=gamma.offset,
                            ap=[[0, 128], gamma.ap[1]]))
            beta_rep = consts.tile([128, D], F32)
            nc.sync.dma_start(
                out=beta_rep,
                in_=bass.AP(tensor=beta.tensor, offset=beta.offset,
                            ap=[[0, 128], beta.ap[1]]))

            idx_sb = consts.tile([128, NCHUNK, CHUNK // 16], I16)
            nc.sync.dma_start(out=idx_sb, in_=idx)

            # E_sb[p, t, d] = emb[t*128 + p, d]
            E_sb = sb.tile([128, NT, 128], F32)
            nc.sync.dma_start(out=E_sb,
                              in_=emb.rearrange("(t p) d -> p t d", p=128))

            # ---- per-128-row-tile column sums S[t, d] ----
            # S_flat[0, t*128 + d] = sum_p E_sb[p, t, d]
            onescol_sb = consts.tile([128, 1], F32)
            nc.vector.memset(onescol_sb, 1.0)
            E_flat = E_sb.rearrange("p t d -> p (t d)")
            S_flat = sb.tile([1, NT * 128], F32)
            for j in range(NJ):
                S_ps = ps_s.tile([1, 512], F32)
                nc.tensor.matmul(
                    S_ps, onescol_sb,
                    E_flat[:, j * 512:(j + 1) * 512],
                    start=True, stop=True)
                nc.scalar.copy(out=S_flat[:, j * 512:(j + 1) * 512], in_=S_ps)
            # redistribute to S_sb[t, d] = S of tile t
            S_sb = sb.tile([64, 128], F32)
            nc.sync.dma_start(
                out=S_sb.rearrange("t d -> t () d"),
                in_=S_flat.rearrange("p (t d) -> p t d", d=128))

            # ---- carries: carry[t, d] = sum_{s < t} S[s, d] ----
            carry_ps = ps_c.tile([64, 128], F32)
            nc.tensor.matmul(carry_ps, su64_sb, S_sb, start=True, stop=True)
            carry_sb = sb.tile([64, 128], F32)
            nc.scalar.copy(out=carry_sb, in_=carry_ps)
            # flatten to one partition: carry_flat[0, t*128+d] = carry[t, d]
            carry_flat = sb.tile([1, NT * 128], F32)
            nc.sync.dma_start(
                out=carry_flat.rearrange("p (t d) -> p t d", d=128),
                in_=carry_sb.rearrange("t d -> t () d"))

            # ---- main pass: cumsum chunks + LayerNorm -> normed table ----
            normed_t = normed.rearrange("(t p) d -> p t d", p=128)
            for j in range(NJ):
                P = ps_p.tile([128, 4, 128], F32)
                P_flat = P.rearrange("p u d -> p (u d)")
                nc.tensor.matmul(
                    P_flat, triu_sb, E_flat[:, j * 512:(j + 1) * 512],
                    start=True, stop=False)
                nc.tensor.matmul(
                    P_flat, ones1_sb,
                    carry_flat[:, j * 512:(j + 1) * 512],
                    start=False, stop=True)

                st = stats.tile([128, 4, 6], F32)
                mv = stats.tile([128, 4, 2], F32)
                for u in range(4):
                    nc.vector.bn_stats(out=st[:, u, :], in_=P[:, u, :])
                    nc.vector.bn_aggr(out=mv[:, u, :], in_=st[:, u, :])
                # rstd = 1/sqrt(var + eps)
                rstd = stats.tile([128, 4], F32)
                nc.scalar.activation(
                    out=rstd, in_=mv[:, :, 1],
                    func=mybir.ActivationFunctionType.Sqrt,
                    bias=eps_sb, scale=1.0)
                nc.vector.reciprocal(out=rstd, in_=rstd)
                # negmurstd = -mu * rstd
                nmr = stats.tile([128, 4], F32)
                nc.vector.scalar_tensor_tensor(
                    out=nmr, in0=mv[:, :, 0], scalar=-1.0, in1=rstd,
                    op0=mybir.AluOpType.mult, op1=mybir.AluOpType.mult)

                Y = ysb.tile([128, 4, 128], F32)
                for u in range(4):
                    nc.scalar.activation(
                        out=Y[:, u, :], in_=P[:, u, :],
                        func=mybir.ActivationFunctionType.Identity,
                        bias=nmr[:, u:u + 1], scale=rstd[:, u:u + 1])
                # gamma / beta (broadcast across the 4 tiles of the chunk)
                g_b = bass.AP(tensor=gamma_rep.tensor, offset=gamma_rep.offset,
                              ap=[gamma_rep.ap[0], [0, 4], gamma_rep.ap[1]])
                b_b = bass.AP(tensor=beta_rep.tensor, offset=beta_rep.offset,
                              ap=[beta_rep.ap[0], [0, 4], beta_rep.ap[1]])
                nc.vector.tensor_mul(Y, Y, g_b)
                nc.vector.tensor_add(Y, Y, b_b)

                nc.sync.dma_start(out=normed_t[:, 4 * j:4 * j + 4, :], in_=Y)

            # all table writes must land before gathers read the table
            tc.strict_bb_all_engine_barrier()

            # ---- gather phase ----
            out_t = out.rearrange("(g p c) d -> g p c d", g=NCHUNK, p=128)
            with tc.tile_pool(name="gat", bufs=8) as gat:
                for g in range(NCHUNK):
                    G = gat.tile([128, CHUNK // 128, D], F32)
                    nc.gpsimd.dma_gather(
                        G, normed, idx_sb[:, g, :], CHUNK, CHUNK, D,
                        single_packet=False, queue_num=g % 4)
                    nc.sync.dma_start(out=out_t[g], in_=G)

    nc.compile()
    return nc


_CACHE = {}


def _get_program():
    if "nc" not in _CACHE:
        _CACHE["nc"] = _build_program()
    return _CACHE["nc"]


def _format_indices(flat_idx: np.ndarray) -> np.ndarray:
    """Per-core index formatting for dma_gather.

    flat_idx: [ROWS_PER_CORE] int array (this core's output rows, in order).
    Returns [128, NCHUNK * CHUNK//16] int16 SBUF image.

    For chunk g (output rows base..base+CHUNK-1):
      - gather list position i = c*128 + p must hold the index for output
        row base + p*(CHUNK//128) + c, so that gathered SBUF[p, c] rows write
        back to DRAM with one fully-contiguous DMA.
      - the SWDGE consumes the list wrapped over 16 partitions
        (idxs[q, s] = list[s*16 + q%16], replicated across the 8 Q7 cores).
    """
    C = CHUNK // 128  # 64
    cols = CHUNK // 16  # 512
    outimg = np.empty((128, NCHUNK * cols), dtype=np.int16)
    for g in range(NCHUNK):
        chunk = flat_idx[g * CHUNK:(g + 1) * CHUNK]
        lst = chunk.reshape(128, C).T.reshape(-1)  # lst[c*128+p]
        w16 = lst.reshape(cols, 16).T.astype(np.int16)  # [16, cols]
        outimg[:, g * cols:(g + 1) * cols] = np.tile(w16, (8, 1))
    return outimg


def kernel(index_tensor, embedding_matrix, ln_gamma, ln_beta):
    index_tensor = np.asarray(index_tensor)
    embedding_matrix = np.asarray(embedding_matrix, dtype=np.float32)
    ln_gamma = np.asarray(ln_gamma, dtype=np.float32).reshape(1, D)
    ln_beta = np.asarray(ln_beta, dtype=np.float32).reshape(1, D)
    idx_dtype = index_tensor.dtype

    triu = np.triu(np.ones((128, 128), dtype=np.float32))
    su64 = np.triu(np.ones((64, 64), dtype=np.float32), k=1)

    nc = _get_program()

    batches_per_core = B // NCORES
    in_maps = []
    for c in range(NCORES):
        shard = index_tensor[c * batches_per_core:(c + 1) * batches_per_core]
        flat = np.ascontiguousarray(shard).reshape(-1).astype(np.int64)
        in_maps.append({
            "emb": embedding_matrix,
            "gamma": ln_gamma,
            "beta": ln_beta,
            "idx": _format_indices(flat),
            "triu": triu,
            "su64": su64,
        })

    res = run_bass_kernel_spmd(nc, in_maps, core_ids=list(range(NCORES)))
    outs = [r["out"].reshape(batches_per_core, S, D) for r in res.results]
    full = np.concatenate(outs, axis=0)
    # keep index dtype untouched on the way out (ints in, floats out)
    del idx_dtype
    return full
